# revision 7
# baseline (speedup 1.0000x reference)
"""Trainium2 Bass kernel for nn_AttentionBlock (B=4, C=256, H=W=64, IC=128).

Sharding: 8 cores = 4 batches x 2 row-halves of the N=4096 attention dim.
Each core computes its 2048 rows of the attention output, the final 1x1 conv
(wy), and partial BatchNorm statistics; a tiny AllReduce combines the BN
stats; each core then applies BN + residual and writes its output slice.

Algebraic simplifications vs the reference (all exact):
  - g_b and w_b only add a per-channel constant to wy, which BatchNorm's
    mean subtraction cancels -> dropped.
  - dy_b (phi bias) only adds row-constant terms to the attention logits,
    which softmax cancels -> dropped. Only dx_b (theta bias) is applied.
  - softmax is computed without max-subtraction: logits are bounded
    (|f| < ~70 for randn inputs), well within f32/bf16 exp range.

Layout: everything is kept channels-on-partitions. The attention scores are
computed TRANSPOSED (fT[m, n] tiles with m on partitions) so that exp(fT)
chunks feed the second matmul (y2 = P @ g) directly as the stationary-K
operand with no transposes. The softmax denominator d[n] = sum_m exp(fT)
is computed on the TensorEngine with a ones-column stationary.
"""

import sys
import numpy as np

if "/opt/trn_rl_repo" not in sys.path:
    sys.path.insert(0, "/opt/trn_rl_repo")

import concourse.bass as bass
import concourse.bacc as bacc
import concourse.mybir as mybir
import concourse.tile as tile
from concourse.bass_utils import run_bass_kernel_spmd

N_CORES = 8
B, C, HW = 4, 256, 64
N = HW * HW          # 4096 spatial positions per batch
IC = 128             # inter channels
NL = N // 2          # 2048 rows per core
NH = NL // 2         # 1024 cols per attention n-half
EPS = 1e-5
CNT = float(B * N)   # BatchNorm count per channel

f32 = mybir.dt.float32
bf16 = mybir.dt.bfloat16
f16 = mybir.dt.float16
ALU = mybir.AluOpType
ACTF = mybir.ActivationFunctionType


def _build():
    nc = bacc.Bacc("TRN2", target_bir_lowering=False, debug=False,
                   num_devices=N_CORES)

    xl_d = nc.dram_tensor("xl", [C, NL], f32, kind="ExternalInput").ap()
    yl_d = nc.dram_tensor("yl", [C, N], f32, kind="ExternalInput").ap()
    dxwT_d = nc.dram_tensor("dxwT", [C, IC], f32, kind="ExternalInput").ap()
    dywT_d = nc.dram_tensor("dywT", [C, IC], f32, kind="ExternalInput").ap()
    gwT_d = nc.dram_tensor("gwT", [C, IC], f32, kind="ExternalInput").ap()
    wwT_d = nc.dram_tensor("wwT", [IC, C], f32, kind="ExternalInput").ap()
    dxb_d = nc.dram_tensor("dxb", [IC, 1], f32, kind="ExternalInput").ap()
    gamma_d = nc.dram_tensor("gamma", [C, 1], f32, kind="ExternalInput").ap()
    beta_d = nc.dram_tensor("beta", [C, 1], f32, kind="ExternalInput").ap()
    out_d = nc.dram_tensor("out", [C, NL], f32, kind="ExternalOutput").ap()

    with tile.TileContext(nc) as tc:
        _emit(nc, tc, xl_d, yl_d, dxwT_d, dywT_d, gwT_d, wwT_d, dxb_d,
              gamma_d, beta_d, out_d)
    nc.compile()
    return nc


def _emit(nc, tc, xl_d, yl_d, dxwT_d, dywT_d, gwT_d, wwT_d, dxb_d,
          gamma_d, beta_d, out_d):
    with (
        tc.tile_pool(name="sb_w", bufs=1) as wp,        # weights + tiny tiles
        tc.tile_pool(name="sb_x", bufs=2) as xp,        # x / y staging
        tc.tile_pool(name="sb_a", bufs=1) as ap_,       # activations (theta/phi/g)
        tc.tile_pool(name="sb_e", bufs=3) as ep,        # exp tiles
        tc.tile_pool(name="sb_m", bufs=2) as mp,        # misc per-half tiles
        tc.tile_pool(name="sb_bn", bufs=1) as bp,       # bn tiny tiles
        tc.tile_pool(name="dram", bufs=1, space="DRAM") as dr,
    ):
        # ---------------- load + cast inputs ----------------
        xl_t, xh_t, yh_t = [], [], []
        for i in range(2):
            xt = xp.tile([128, NL], f32, tag="xl")
            nc.sync.dma_start(xt[:], xl_d[128 * i:128 * (i + 1), :])
            xl_t.append(xt)
            xh = xp.tile([128, NL], f16, tag="xh")
            nc.vector.tensor_copy(xh[:], xt[:])
            xh_t.append(xh)
        for i in range(2):
            yt = xp.tile([128, N], f32, tag="yl")
            nc.sync.dma_start(yt[:], yl_d[128 * i:128 * (i + 1), :])
            yh = xp.tile([128, N], f16, tag="yh")
            nc.vector.tensor_copy(yh[:], yt[:])
            yh_t.append(yh)

        wdx_h, wdy_h, wg_h = [], [], []
        for i in range(2):
            for nm, (dst, src_d) in (("dx", (wdx_h, dxwT_d)),
                                     ("dy", (wdy_h, dywT_d)),
                                     ("g", (wg_h, gwT_d))):
                wt = wp.tile([128, IC], f32, tag="wtmp", bufs=2)
                nc.sync.dma_start(wt[:], src_d[128 * i:128 * (i + 1), :])
                wh = wp.tile([128, IC], f16, tag=f"wh_{nm}{i}")
                nc.vector.tensor_copy(wh[:], wt[:])
                dst.append(wh)
        wwT_f = wp.tile([IC, C], f32, tag="wwT_f")
        nc.sync.dma_start(wwT_f[:], wwT_d[:])
        wwT_b = wp.tile([IC, C], bf16, tag="wwT_b")
        nc.vector.tensor_copy(wwT_b[:], wwT_f[:])

        dxb_t = wp.tile([IC, 1], f32, tag="dxb")
        nc.sync.dma_start(dxb_t[:], dxb_d[:])
        gamma_t, beta_t = [], []
        for i in range(2):
            gt = wp.tile([128, 1], f32, tag=f"gam{i}")
            nc.sync.dma_start(gt[:], gamma_d[128 * i:128 * (i + 1), :])
            gamma_t.append(gt)
            bt = wp.tile([128, 1], f32, tag=f"bet{i}")
            nc.sync.dma_start(bt[:], beta_d[128 * i:128 * (i + 1), :])
            beta_t.append(bt)

        ones_m = wp.tile([128, 1], bf16, tag="ones_m")   # d-matmul stationary
        nc.vector.memset(ones_m[:], 1.0)
        ones_r = wp.tile([1, 128], f32, tag="ones_r")    # rinv broadcast stationary
        nc.vector.memset(ones_r[:], 1.0)

        # ---------------- projections (PSUM phase 1) ----------------
        theta_h = ap_.tile([IC, NL], f16, tag="theta")
        phi_h = ap_.tile([IC, N], f16, tag="phi")
        g_sb = ap_.tile([128, N], bf16, tag="g")   # 32 chunks [m128, ic128]

        with tc.tile_pool(name="ps_proj", bufs=2, space="PSUM") as pp:
            # g projection: chunk m -> g[m128, ic] = sum_c y[c, m128].T @ gwT[c, ic]
            for t in range(2):                  # two psum tiles, 16 m-chunks each
                gp = pp.tile([128, 2048], f32, tag="proj")
                for j in range(16):
                    m = 16 * t + j
                    for i in range(2):
                        nc.tensor.matmul(
                            gp[:, 128 * j:128 * (j + 1)],
                            yh_t[i][:, 128 * m:128 * (m + 1)],
                            wg_h[i][:],
                            start=(i == 0), stop=(i == 1))
                nc.scalar.copy(g_sb[:, 2048 * t:2048 * (t + 1)], gp[:])

            # theta: [ic, nl] = dxwT.T @ x  (+ dx_b)
            tp = pp.tile([128, 2048], f32, tag="proj")
            for j in range(4):
                for i in range(2):
                    nc.tensor.matmul(
                        tp[:, 512 * j:512 * (j + 1)],
                        wdx_h[i][:],
                        xh_t[i][:, 512 * j:512 * (j + 1)],
                        start=(i == 0), stop=(i == 1))
            nc.vector.tensor_scalar(theta_h[:], tp[:], dxb_t[:], None, ALU.add)

            # phi: [ic, n] = dywT.T @ y  (bias dropped: softmax-invariant)
            for h in range(2):
                php = pp.tile([128, 2048], f32, tag="proj")
                for j in range(4):
                    for i in range(2):
                        nc.tensor.matmul(
                            php[:, 512 * j:512 * (j + 1)],
                            wdy_h[i][:],
                            yh_t[i][:, 2048 * h + 512 * j:2048 * h + 512 * (j + 1)],
                            start=(i == 0), stop=(i == 1))
                nc.vector.tensor_copy(phi_h[:, 2048 * h:2048 * (h + 1)], php[:])

        # ---------------- attention (PSUM phase 2) ----------------
        wy_sb = [mp.tile([128, NL], f32, tag=f"wy{c}", bufs=1, name=f"wy_sb{c}")
                 for c in range(2)]
        s_t = [[None] * 2 for _ in range(2)]   # [c][half] partial sums
        q_t = [[None] * 2 for _ in range(2)]   # [c][half] partial sumsq

        with tc.tile_pool(name="ps_attn", bufs=1, space="PSUM") as pa:
            for h2 in range(2):
                n0 = NH * h2
                y2_ps = pa.tile([IC, NH], f32, tag="y2")
                d_ps = pa.tile([1, NH], f32, tag="d")
                for m in range(32):
                    ft = pa.tile([128, NH], f32, tag="ft", bufs=2)
                    for j in range(2):
                        nc.tensor.matmul(
                            ft[:, 512 * j:512 * (j + 1)],
                            phi_h[:, 128 * m:128 * (m + 1)],
                            theta_h[:, n0 + 512 * j:n0 + 512 * (j + 1)],
                            start=True, stop=True)
                    expP = ep.tile([128, NH], bf16, tag="exp")
                    nc.scalar.activation(expP[:], ft[:], ACTF.Exp)
                    for j in range(2):
                        nc.tensor.matmul(
                            y2_ps[:, 512 * j:512 * (j + 1)],
                            g_sb[:, 128 * m:128 * (m + 1)],
                            expP[:, 512 * j:512 * (j + 1)],
                            start=(m == 0), stop=(m == 31))
                    for j in range(2):
                        nc.tensor.matmul(
                            d_ps[:, 512 * j:512 * (j + 1)],
                            ones_m[:],
                            expP[:, 512 * j:512 * (j + 1)],
                            start=(m == 0), stop=(m == 31))

                # normalize: y2sb = y2 / d   (broadcast 1/d via PE ones-matmul)
                rinv = mp.tile([1, NH], f32, tag="rinv")
                nc.vector.reciprocal(rinv[:], d_ps[:])
                rb_ps = pa.tile([128, NH], f32, tag="ft", bufs=2)
                for j in range(2):
                    nc.tensor.matmul(
                        rb_ps[:, 512 * j:512 * (j + 1)],
                        ones_r[:],
                        rinv[:, 512 * j:512 * (j + 1)],
                        start=True, stop=True)
                rb_sb = mp.tile([128, NH], f32, tag="rb")
                nc.vector.tensor_copy(rb_sb[:], rb_ps[:])
                y2sb = mp.tile([IC, NH], bf16, tag="y2sb")
                nc.vector.tensor_tensor(y2sb[:], y2_ps[:], rb_sb[:], op=ALU.mult)

                # wy = wwT.T @ y2sb ; per-channel partial stats via accum_out
                for c in range(2):
                    wyp = pa.tile([128, NH], f32, tag="ft", bufs=2)
                    for j in range(2):
                        nc.tensor.matmul(
                            wyp[:, 512 * j:512 * (j + 1)],
                            wwT_b[:, 128 * c:128 * (c + 1)],
                            y2sb[:, 512 * j:512 * (j + 1)],
                            start=True, stop=True)
                    s = bp.tile([128, 1], f32, tag=f"s{c}{h2}")
                    q = bp.tile([128, 1], f32, tag=f"q{c}{h2}")
                    nc.scalar.activation(wy_sb[c][:, n0:n0 + NH], wyp[:],
                                         ACTF.Copy, accum_out=s[:])
                    sq = ep.tile([128, NH], bf16, tag="sqscratch")
                    nc.scalar.activation(sq[:], wyp[:], ACTF.Square,
                                         accum_out=q[:])
                    s_t[c][h2], q_t[c][h2] = s, q

        # ---------------- BN stats AllReduce ----------------
        packed = bp.tile([128, 4], f32, tag="packed")
        for c in range(2):
            nc.vector.tensor_tensor(packed[:, 2 * c:2 * c + 1],
                                    s_t[c][0][:], s_t[c][1][:], op=ALU.add)
            nc.vector.tensor_tensor(packed[:, 2 * c + 1:2 * c + 2],
                                    q_t[c][0][:], q_t[c][1][:], op=ALU.add)
        ar_in = dr.tile([128, 4], f32)
        ar_out = dr.tile([128, 4], f32)
        nc.gpsimd.dma_start(ar_in[:], packed[:])
        nc.gpsimd.collective_compute(
            "AllReduce", ALU.add,
            replica_groups=[list(range(N_CORES))],
            ins=[ar_in.opt()], outs=[ar_out.opt()])
        stats_g = bp.tile([128, 4], f32, tag="stats_g")
        nc.gpsimd.dma_start(stats_g[:], ar_out[:])

        # ---------------- BN apply + residual ----------------
        for c in range(2):
            mean = bp.tile([128, 1], f32, tag=f"mean{c}")
            nc.vector.tensor_scalar(mean[:], stats_g[:, 2 * c:2 * c + 1],
                                    1.0 / CNT, None, ALU.mult)
            msq = bp.tile([128, 1], f32, tag=f"msq{c}")
            nc.vector.tensor_scalar(msq[:], stats_g[:, 2 * c + 1:2 * c + 2],
                                    1.0 / CNT, None, ALU.mult)
            m2 = bp.tile([128, 1], f32, tag=f"m2{c}")
            nc.vector.tensor_tensor(m2[:], mean[:], mean[:], op=ALU.mult)
            var = bp.tile([128, 1], f32, tag=f"var{c}")
            nc.vector.tensor_tensor(var[:], msq[:], m2[:], op=ALU.subtract)
            varep = bp.tile([128, 1], f32, tag=f"varep{c}")
            nc.vector.tensor_scalar(varep[:], var[:], float(EPS), None, ALU.add)
            sd = bp.tile([128, 1], f32, tag=f"sd{c}")
            nc.scalar.activation(sd[:], varep[:], ACTF.Sqrt)
            rstd = bp.tile([128, 1], f32, tag=f"rstd{c}")
            nc.vector.reciprocal(rstd[:], sd[:])
            scale = bp.tile([128, 1], f32, tag=f"scale{c}")
            nc.vector.tensor_tensor(scale[:], gamma_t[c][:], rstd[:], op=ALU.mult)
            msc = bp.tile([128, 1], f32, tag=f"msc{c}")
            nc.vector.tensor_tensor(msc[:], mean[:], scale[:], op=ALU.mult)
            shift = bp.tile([128, 1], f32, tag=f"shift{c}")
            nc.vector.tensor_tensor(shift[:], beta_t[c][:], msc[:], op=ALU.subtract)

            out_t = mp.tile([128, NL], f32, tag=f"out{c}", bufs=1)
            nc.vector.tensor_scalar(out_t[:], wy_sb[c][:], scale[:], shift[:],
                                    ALU.mult, ALU.add)
            nc.vector.tensor_tensor(out_t[:], out_t[:], xl_t[c][:], op=ALU.add)
            nc.sync.dma_start(out_d[128 * c:128 * (c + 1), :], out_t[:])


_NC_CACHE = None


def _get_nc():
    global _NC_CACHE
    if _NC_CACHE is None:
        _NC_CACHE = _build()
    return _NC_CACHE


def shard_inputs(inputs):
    x = np.ascontiguousarray(inputs["x"], dtype=np.float32).reshape(B, C, N)
    y = np.ascontiguousarray(inputs["y"], dtype=np.float32).reshape(B, C, N)
    dxwT = np.ascontiguousarray(inputs["dx_w"].T, dtype=np.float32)
    dywT = np.ascontiguousarray(inputs["dy_w"].T, dtype=np.float32)
    gwT = np.ascontiguousarray(inputs["g_w"].T, dtype=np.float32)
    wwT = np.ascontiguousarray(inputs["w_w"].T, dtype=np.float32)
    dxb = np.ascontiguousarray(inputs["dx_b"], dtype=np.float32).reshape(IC, 1)
    gamma = np.ascontiguousarray(inputs["bn_gamma"], dtype=np.float32).reshape(C, 1)
    beta = np.ascontiguousarray(inputs["bn_beta"], dtype=np.float32).reshape(C, 1)

    in_maps = []
    for core in range(N_CORES):
        b, h = divmod(core, 2)
        in_maps.append({
            "xl": np.ascontiguousarray(x[b][:, h * NL:(h + 1) * NL]),
            "yl": y[b],
            "dxwT": dxwT, "dywT": dywT, "gwT": gwT, "wwT": wwT,
            "dxb": dxb, "gamma": gamma, "beta": beta,
        })
    return in_maps


def run(inputs, **kw):
    """Run on hardware; returns (full_output, BassKernelResults)."""
    nc = _get_nc()
    in_maps = shard_inputs(inputs)
    r = run_bass_kernel_spmd(nc, in_maps, core_ids=list(range(N_CORES)), **kw)
    out = np.empty((B, C, N), np.float32)
    for core in range(N_CORES):
        b, h = divmod(core, 2)
        out[b][:, h * NL:(h + 1) * NL] = r.results[core]["out"]
    return out.reshape(B, C, HW, HW), r


def kernel(**inputs):
    out, _ = run(inputs)
    return out
